# revision 9
# baseline (speedup 1.0000x reference)
"""Trainium2 Bass kernel for nn_DebedderNeuronGroup_index.

Math (per layer l, with kn=KN[l], ksci=KS[l]*CI[l], i_dim=ksci+1):
    out[b, k, o] = sum_d x[b, off_l + k, d] * W_l[o, d] + b_l[o]
    y[b, S_l + k*ksci + o] = out[b, k, o]          for o <  ksci
    y[b, S_l + kn*ksci + k] = out[b, k, ksci]      (bias column tail block)
The five layers' outputs exactly tile y's 1,422,218 columns, so every
element of y is written exactly once (pure permutation, no accumulation).

Strategy: pure data parallelism over batch (16 per core, 8 cores).
Host pre-transposes x to xT[d, token] (token order layer-major then
batch-major) and W to WT[d, o], both cast to bf16 (matmul runs 4x faster
than fp32 on the PE; rel err ~5e-4). On device, per 128-token tile:
tokens sit on PSUM partitions (stationary operand = xT tile), o on the
free dim, so every HBM store is a [tokens, o] tile whose rows are
contiguous runs in y. Bias is added during the PSUM->SBUF drain with a
host-replicated [128, ksci] broadcast table. The bias column (o == ksci)
is computed in a separate tiny pass with M=1 matmuls producing [1, token]
rows that store contiguously into the tail blocks.
"""

import numpy as np
import ml_dtypes

import concourse.bass as bass
import concourse.mybir as mybir
from concourse import bacc
from concourse.tile import TileContext
from concourse.bass_utils import run_bass_kernel_spmd

# ---------------------------------------------------------------- constants
N_CORES = 8
B = 128
BPC = B // N_CORES            # batches per core = 16
D = 512
KN = [64, 128, 256, 256, 10]
KSCI = [27, 576, 1152, 4096, 256]
IDIM = [k + 1 for k in KSCI]
START = [0, 1792, 75648, 370816, 1419648]
I_TOTAL = 1422218
TOK = sum(KN)                 # 714 tokens per batch
TOKL = [BPC * k for k in KN]  # tokens per core per layer
XOFF = np.cumsum([0] + TOKL).tolist()   # token offset per layer in xT
NTOK = XOFF[-1]               # 11424
BBOFF = np.cumsum([0] + KSCI).tolist()  # bias-broadcast offset per layer
BBTOT = BBOFF[-1]             # 6107
TLOAD = 1024                  # tokens per x DMA chunk
OTILE = 512                   # matmul moving free dim / PSUM bank
BF16 = mybir.dt.bfloat16
F16 = mybir.dt.float16
F32 = mybir.dt.float32

_cache = {}
last_results = None


def _build_bass():
    nc = bacc.Bacc(
        "TRN2", target_bir_lowering=False, debug=False, num_devices=N_CORES
    )
    xT = nc.declare_dram_parameter("xT", [D, NTOK], BF16, isOutput=False)
    WT = [
        nc.declare_dram_parameter(f"WT{l}", [D, IDIM[l]], BF16, isOutput=False)
        for l in range(5)
    ]
    BB = nc.declare_dram_parameter("BB", [128, BBTOT], F32, isOutput=False)
    BCOL = nc.declare_dram_parameter("BCOL", [1, 8], F32, isOutput=False)
    y = nc.declare_dram_parameter("y", [BPC, I_TOTAL], F16, isOutput=True)

    xT3 = xT[:, :].rearrange("(c p) t -> p c t", p=128)      # [128, 4, NTOK]

    with TileContext(nc) as tc:
        with (
            tc.tile_pool(name="wt", bufs=1) as wt_pool,
            tc.tile_pool(name="bias", bufs=1) as bias_pool,
            tc.tile_pool(name="x", bufs=3) as x_pool,
            tc.tile_pool(name="out", bufs=3) as out_pool,
            tc.tile_pool(name="ocol", bufs=2) as ocol_pool,
            tc.tile_pool(name="ps", bufs=4, space="PSUM") as ps_pool,
            tc.tile_pool(name="pscol", bufs=2, space="PSUM") as pscol_pool,
        ):
            # Tables are loaded just-in-time per layer (first matmul would
            # otherwise stall ~35us behind 9.4 MB of upfront table DMAs).
            bb = bias_pool.tile([128, BBTOT], F32, tag="bb")
            bcol = bias_pool.tile([1, 8], F32, tag="bcol")
            nc.sync.dma_start(out=bcol[:], in_=BCOL[:, :])

            # Layer 4 early (its tiny strided stores hide under compute);
            # layer 3 last (largest, most efficient stores stream the tail).
            for l in [0, 4, 1, 2, 3]:
                t = wt_pool.tile([128, 4 * IDIM[l]], BF16, tag=f"wt{l}")
                wt3_l = t[:].rearrange("p (c o) -> p c o", c=4)
                nc.sync.dma_start(
                    out=wt3_l, in_=WT[l][:, :].rearrange("(c p) o -> p c o", p=128)
                )
                nc.sync.dma_start(
                    out=bb[:, BBOFF[l] : BBOFF[l] + KSCI[l]],
                    in_=BB[:, BBOFF[l] : BBOFF[l] + KSCI[l]],
                )
                kn, ksci = KN[l], KSCI[l]
                # y main region viewed [b, k, o]; tail region viewed [b, k]
                y_main = y[:, START[l] : START[l] + kn * ksci].rearrange(
                    "b (k o) -> b k o", o=ksci
                )
                y_col = y[:, START[l] + kn * ksci : START[l] + kn * ksci + kn]
                # subtile = whole batches when kn < 128, else 128-token slice
                ts = 128 if kn >= 128 else (128 // kn) * kn
                for t0 in range(0, TOKL[l], TLOAD):
                    tl = min(TLOAD, TOKL[l] - t0)
                    xt = x_pool.tile([128, 4 * TLOAD], BF16, tag="xt")
                    xt3 = xt[:].rearrange("p (c t) -> p c t", c=4)
                    nc.sync.dma_start(
                        out=xt3[:, :, :tl],
                        in_=xT3[:, :, XOFF[l] + t0 : XOFF[l] + t0 + tl],
                    )
                    # ---- main pass: tokens on partitions, o on free dim.
                    # All o-tiles of a token-subtile drain into one wide SBUF
                    # tile so each store DMA writes full ksci-long rows
                    # (8 KB runs for layer 3 instead of 1 KB per o-tile).
                    for s0 in range(0, tl, ts):
                        sl = min(ts, tl - s0)         # tokens in subtile
                        tok = t0 + s0                  # layer-token index
                        b0 = tok // kn                 # first batch
                        nb = max(1, sl // kn)          # batches in subtile
                        k0 = tok - b0 * kn             # first k (0 unless kn>128... )
                        ob = out_pool.tile([128, 4096], F16, tag="ob")
                        for o0 in range(0, ksci, OTILE):
                            no = min(OTILE, ksci - o0)
                            ps = ps_pool.tile([128, OTILE], F32, tag="ps")
                            for dc in range(4):
                                nc.tensor.matmul(
                                    out=ps[:sl, :no],
                                    lhsT=xt3[:, dc, s0 : s0 + sl],
                                    rhs=wt3_l[:, dc, o0 : o0 + no],
                                    start=(dc == 0),
                                    stop=(dc == 3),
                                )
                            nc.any.tensor_add(
                                out=ob[:sl, o0 : o0 + no],
                                in0=ps[:sl, :no],
                                in1=bb[:sl, BBOFF[l] + o0 : BBOFF[l] + o0 + no],
                            )
                        # store per batch: [nk, ksci] rows contiguous in y
                        nk = min(kn, sl)
                        for bi in range(nb):
                            nc.sync.dma_start(
                                out=y_main[b0 + bi, k0 : k0 + nk, :],
                                in_=ob[bi * nk : bi * nk + nk, :ksci],
                            )
                    # ---- bias-column pass: [1, token] rows
                    for c0 in range(0, tl, OTILE):
                        cl = min(OTILE, tl - c0)
                        pc = pscol_pool.tile([1, OTILE], F32, tag="pc")
                        for dc in range(4):
                            nc.tensor.matmul(
                                out=pc[:1, :cl],
                                lhsT=wt3_l[:, dc, ksci : ksci + 1],
                                rhs=xt3[:, dc, c0 : c0 + cl],
                                start=(dc == 0),
                                stop=(dc == 3),
                            )
                        oc = ocol_pool.tile([1, OTILE], F16, tag="oc")
                        nc.any.tensor_scalar_add(
                            out=oc[:1, :cl],
                            in0=pc[:1, :cl],
                            scalar1=bcol[0:1, l : l + 1],
                        )
                        # tokens (t0+c0 .. +cl) are whole batches here
                        cb0 = (t0 + c0) // kn
                        cnb = cl // kn
                        for bi in range(cnb):
                            nc.sync.dma_start(
                                out=y_col[cb0 + bi, :],
                                in_=oc[0:1, bi * kn : (bi + 1) * kn],
                            )
    nc.compile()
    return nc


def _prep_inputs(inputs):
    x = np.asarray(inputs["x"], dtype=np.float32)
    xb = x.astype(ml_dtypes.bfloat16)
    in_maps = []
    # shared across cores
    shared = {}
    for l in range(5):
        W = np.asarray(inputs[f"W{l}"], dtype=np.float32)
        shared[f"WT{l}"] = np.ascontiguousarray(W.astype(ml_dtypes.bfloat16).T)
    bbvec = np.concatenate(
        [np.asarray(inputs[f"b{l}"], dtype=np.float32)[: KSCI[l]] for l in range(5)]
    )
    shared["BB"] = np.ascontiguousarray(np.broadcast_to(bbvec, (128, BBTOT)))
    bcol = np.zeros((1, 8), np.float32)
    for l in range(5):
        bcol[0, l] = np.asarray(inputs[f"b{l}"], dtype=np.float32)[KSCI[l]]
    shared["BCOL"] = bcol
    off = np.cumsum([0] + KN).tolist()
    for c in range(N_CORES):
        xc = xb[c * BPC : (c + 1) * BPC]  # [16, 714, 512] bf16
        parts = [
            np.transpose(xc[:, off[l] : off[l] + KN[l]], (2, 0, 1)).reshape(D, -1)
            for l in range(5)
        ]
        xT = np.ascontiguousarray(np.concatenate(parts, axis=1))  # [512, 11424]
        in_maps.append({"xT": xT, **shared})
    return in_maps


def kernel(**inputs):
    global last_results
    if "nc" not in _cache:
        _cache["nc"] = _build_bass()
    nc = _cache["nc"]
    in_maps = _prep_inputs(inputs)
    res = run_bass_kernel_spmd(nc, in_maps, list(range(N_CORES)))
    last_results = res
    y = np.concatenate(
        [res.results[c]["y"].astype(np.float32) for c in range(N_CORES)], axis=0
    )
    return y


# revision 10
# speedup vs baseline: 1.0571x; 1.0571x over previous
"""Trainium2 Bass kernel for nn_DebedderNeuronGroup_index.

Math (per layer l, with kn=KN[l], ksci=KS[l]*CI[l], i_dim=ksci+1):
    out[b, k, o] = sum_d x[b, off_l + k, d] * W_l[o, d] + b_l[o]
    y[b, S_l + k*ksci + o] = out[b, k, o]          for o <  ksci
    y[b, S_l + kn*ksci + k] = out[b, k, ksci]      (bias column tail block)
The five layers' outputs exactly tile y's 1,422,218 columns, so every
element of y is written exactly once (pure permutation, no accumulation).

Strategy: pure data parallelism over batch (16 per core, 8 cores).
Host pre-transposes x to xT[d, token] (token order layer-major then
batch-major) and W to WT[d, o], both cast to bf16 (matmul runs 4x faster
than fp32 on the PE; rel err ~5e-4). On device, per 128-token tile:
tokens sit on PSUM partitions (stationary operand = xT tile), o on the
free dim, so every HBM store is a [tokens, o] tile whose rows are
contiguous runs in y. Bias is added during the PSUM->SBUF drain with a
host-replicated [128, ksci] broadcast table. The bias column (o == ksci)
is computed in a separate tiny pass with M=1 matmuls producing [1, token]
rows that store contiguously into the tail blocks.
"""

import numpy as np
import ml_dtypes

import concourse.bass as bass
import concourse.mybir as mybir
from concourse import bacc
from concourse.tile import TileContext
from concourse.bass_utils import run_bass_kernel_spmd

# ---------------------------------------------------------------- constants
N_CORES = 8
B = 128
BPC = B // N_CORES            # batches per core = 16
D = 512
KN = [64, 128, 256, 256, 10]
KSCI = [27, 576, 1152, 4096, 256]
IDIM = [k + 1 for k in KSCI]
START = [0, 1792, 75648, 370816, 1419648]
I_TOTAL = 1422218
TOK = sum(KN)                 # 714 tokens per batch
TOKL = [BPC * k for k in KN]  # tokens per core per layer
XOFF = np.cumsum([0] + TOKL).tolist()   # token offset per layer in xT
NTOK = XOFF[-1]               # 11424
BBOFF = np.cumsum([0] + KSCI).tolist()  # bias-broadcast offset per layer
BBTOT = BBOFF[-1]             # 6107
TLOAD = 1024                  # tokens per x DMA chunk
OTILE = 512                   # matmul moving free dim / PSUM bank
BF16 = mybir.dt.bfloat16
F16 = mybir.dt.float16
F32 = mybir.dt.float32

_cache = {}
last_results = None


def _build_bass():
    nc = bacc.Bacc(
        "TRN2", target_bir_lowering=False, debug=False, num_devices=N_CORES
    )
    xT = nc.declare_dram_parameter("xT", [D, NTOK], BF16, isOutput=False)
    WT = [
        nc.declare_dram_parameter(f"WT{l}", [D, IDIM[l]], BF16, isOutput=False)
        for l in range(5)
    ]
    BB = nc.declare_dram_parameter("BB", [128, BBTOT], F32, isOutput=False)
    BCOL = nc.declare_dram_parameter("BCOL", [1, 8], F32, isOutput=False)
    y = nc.declare_dram_parameter("y", [BPC, I_TOTAL], F16, isOutput=True)

    xT3 = xT[:, :].rearrange("(c p) t -> p c t", p=128)      # [128, 4, NTOK]

    with TileContext(nc) as tc:
        with (
            tc.tile_pool(name="wt", bufs=1) as wt_pool,
            tc.tile_pool(name="bias", bufs=1) as bias_pool,
            tc.tile_pool(name="x", bufs=3) as x_pool,
            tc.tile_pool(name="out", bufs=3) as out_pool,
            tc.tile_pool(name="ocol", bufs=2) as ocol_pool,
            tc.tile_pool(name="ps", bufs=4, space="PSUM") as ps_pool,
            tc.tile_pool(name="pscol", bufs=2, space="PSUM") as pscol_pool,
        ):
            # Tables are loaded just-in-time per layer (first matmul would
            # otherwise stall ~35us behind 9.4 MB of upfront table DMAs).
            bb = bias_pool.tile([128, BBTOT], F32, tag="bb")
            bcol = bias_pool.tile([1, 8], F32, tag="bcol")
            nc.sync.dma_start(out=bcol[:], in_=BCOL[:, :])

            # Layer 4 early (its tiny strided stores hide under compute);
            # layer 3 last (largest, most efficient stores stream the tail).
            for l in [0, 4, 1, 2, 3]:
                t = wt_pool.tile([128, 4 * IDIM[l]], BF16, tag=f"wt{l}")
                wt3_l = t[:].rearrange("p (c o) -> p c o", c=4)
                nc.sync.dma_start(
                    out=wt3_l, in_=WT[l][:, :].rearrange("(c p) o -> p c o", p=128)
                )
                nc.sync.dma_start(
                    out=bb[:, BBOFF[l] : BBOFF[l] + KSCI[l]],
                    in_=BB[:, BBOFF[l] : BBOFF[l] + KSCI[l]],
                )
                kn, ksci = KN[l], KSCI[l]
                # y main region viewed [b, k, o]; tail region viewed [b, k]
                y_main = y[:, START[l] : START[l] + kn * ksci].rearrange(
                    "b (k o) -> b k o", o=ksci
                )
                y_col = y[:, START[l] + kn * ksci : START[l] + kn * ksci + kn]
                # subtile = whole batches when kn < 128, else 128-token slice
                ts = 128 if kn >= 128 else (128 // kn) * kn
                for t0 in range(0, TOKL[l], TLOAD):
                    tl = min(TLOAD, TOKL[l] - t0)
                    xt = x_pool.tile([128, 4 * TLOAD], BF16, tag="xt")
                    xt3 = xt[:].rearrange("p (c t) -> p c t", c=4)
                    nc.sync.dma_start(
                        out=xt3[:, :, :tl],
                        in_=xT3[:, :, XOFF[l] + t0 : XOFF[l] + t0 + tl],
                    )
                    # ---- main pass: tokens on partitions, o on free dim.
                    # All o-tiles of a token-subtile drain into one wide SBUF
                    # tile so each store DMA writes full ksci-long rows
                    # (8 KB runs for layer 3 instead of 1 KB per o-tile).
                    for s0 in range(0, tl, ts):
                        sl = min(ts, tl - s0)         # tokens in subtile
                        tok = t0 + s0                  # layer-token index
                        b0 = tok // kn                 # first batch
                        nb = max(1, sl // kn)          # batches in subtile
                        k0 = tok - b0 * kn             # first k (0 unless kn>128... )
                        ob = out_pool.tile([128, 4096], F16, tag="ob")
                        for o0 in range(0, ksci, OTILE):
                            no = min(OTILE, ksci - o0)
                            ps = ps_pool.tile([128, OTILE], F32, tag="ps")
                            for dc in range(4):
                                nc.tensor.matmul(
                                    out=ps[:sl, :no],
                                    lhsT=xt3[:, dc, s0 : s0 + sl],
                                    rhs=wt3_l[:, dc, o0 : o0 + no],
                                    start=(dc == 0),
                                    stop=(dc == 3),
                                )
                            nc.any.tensor_add(
                                out=ob[:sl, o0 : o0 + no],
                                in0=ps[:sl, :no],
                                in1=bb[:sl, BBOFF[l] + o0 : BBOFF[l] + o0 + no],
                            )
                        # store per batch: [nk, ksci] rows contiguous in y
                        nk = min(kn, sl)
                        for bi in range(nb):
                            nc.scalar.dma_start(
                                out=y_main[b0 + bi, k0 : k0 + nk, :],
                                in_=ob[bi * nk : bi * nk + nk, :ksci],
                            )
                    # ---- bias-column pass: [1, token] rows
                    for c0 in range(0, tl, OTILE):
                        cl = min(OTILE, tl - c0)
                        pc = pscol_pool.tile([1, OTILE], F32, tag="pc")
                        for dc in range(4):
                            nc.tensor.matmul(
                                out=pc[:1, :cl],
                                lhsT=wt3_l[:, dc, ksci : ksci + 1],
                                rhs=xt3[:, dc, c0 : c0 + cl],
                                start=(dc == 0),
                                stop=(dc == 3),
                            )
                        oc = ocol_pool.tile([1, OTILE], F16, tag="oc")
                        nc.any.tensor_scalar_add(
                            out=oc[:1, :cl],
                            in0=pc[:1, :cl],
                            scalar1=bcol[0:1, l : l + 1],
                        )
                        # tokens (t0+c0 .. +cl) are whole batches here
                        cb0 = (t0 + c0) // kn
                        cnb = cl // kn
                        for bi in range(cnb):
                            nc.scalar.dma_start(
                                out=y_col[cb0 + bi, :],
                                in_=oc[0:1, bi * kn : (bi + 1) * kn],
                            )
    nc.compile()
    return nc


def _prep_inputs(inputs):
    x = np.asarray(inputs["x"], dtype=np.float32)
    xb = x.astype(ml_dtypes.bfloat16)
    in_maps = []
    # shared across cores
    shared = {}
    for l in range(5):
        W = np.asarray(inputs[f"W{l}"], dtype=np.float32)
        shared[f"WT{l}"] = np.ascontiguousarray(W.astype(ml_dtypes.bfloat16).T)
    bbvec = np.concatenate(
        [np.asarray(inputs[f"b{l}"], dtype=np.float32)[: KSCI[l]] for l in range(5)]
    )
    shared["BB"] = np.ascontiguousarray(np.broadcast_to(bbvec, (128, BBTOT)))
    bcol = np.zeros((1, 8), np.float32)
    for l in range(5):
        bcol[0, l] = np.asarray(inputs[f"b{l}"], dtype=np.float32)[KSCI[l]]
    shared["BCOL"] = bcol
    off = np.cumsum([0] + KN).tolist()
    for c in range(N_CORES):
        xc = xb[c * BPC : (c + 1) * BPC]  # [16, 714, 512] bf16
        parts = [
            np.transpose(xc[:, off[l] : off[l] + KN[l]], (2, 0, 1)).reshape(D, -1)
            for l in range(5)
        ]
        xT = np.ascontiguousarray(np.concatenate(parts, axis=1))  # [512, 11424]
        in_maps.append({"xT": xT, **shared})
    return in_maps


def kernel(**inputs):
    global last_results
    if "nc" not in _cache:
        _cache["nc"] = _build_bass()
    nc = _cache["nc"]
    in_maps = _prep_inputs(inputs)
    res = run_bass_kernel_spmd(nc, in_maps, list(range(N_CORES)))
    last_results = res
    y = np.concatenate(
        [res.results[c]["y"].astype(np.float32) for c in range(N_CORES)], axis=0
    )
    return y


# revision 11
# speedup vs baseline: 1.0860x; 1.0274x over previous
"""Trainium2 Bass kernel for nn_DebedderNeuronGroup_index.

Math (per layer l, with kn=KN[l], ksci=KS[l]*CI[l], i_dim=ksci+1):
    out[b, k, o] = sum_d x[b, off_l + k, d] * W_l[o, d] + b_l[o]
    y[b, S_l + k*ksci + o] = out[b, k, o]          for o <  ksci
    y[b, S_l + kn*ksci + k] = out[b, k, ksci]      (bias column tail block)
The five layers' outputs exactly tile y's 1,422,218 columns, so every
element of y is written exactly once (pure permutation, no accumulation).

Strategy: pure data parallelism over batch (16 per core, 8 cores).
Host pre-transposes x to xT[d, token] (token order layer-major then
batch-major) and W to WT[d, o], both cast to bf16 (matmul runs 4x faster
than fp32 on the PE; rel err ~5e-4). On device, per 128-token tile:
tokens sit on PSUM partitions (stationary operand = xT tile), o on the
free dim, so every HBM store is a [tokens, o] tile whose rows are
contiguous runs in y. Bias is added during the PSUM->SBUF drain with a
host-replicated [128, ksci] broadcast table. The bias column (o == ksci)
is computed in a separate tiny pass with M=1 matmuls producing [1, token]
rows that store contiguously into the tail blocks.
"""

import numpy as np
import ml_dtypes

import concourse.bass as bass
import concourse.mybir as mybir
from concourse import bacc
from concourse.tile import TileContext
from concourse.bass_utils import run_bass_kernel_spmd

# ---------------------------------------------------------------- constants
N_CORES = 8
B = 128
BPC = B // N_CORES            # batches per core = 16
D = 512
KN = [64, 128, 256, 256, 10]
KSCI = [27, 576, 1152, 4096, 256]
IDIM = [k + 1 for k in KSCI]
START = [0, 1792, 75648, 370816, 1419648]
I_TOTAL = 1422218
TOK = sum(KN)                 # 714 tokens per batch
TOKL = [BPC * k for k in KN]  # tokens per core per layer
XOFF = np.cumsum([0] + TOKL).tolist()   # token offset per layer in xT
NTOK = XOFF[-1]               # 11424
BBOFF = np.cumsum([0] + KSCI).tolist()  # bias-broadcast offset per layer
BBTOT = BBOFF[-1]             # 6107
TLOAD = 1024                  # tokens per x DMA chunk
OTILE = 512                   # matmul moving free dim / PSUM bank
BF16 = mybir.dt.bfloat16
F16 = mybir.dt.float16
F32 = mybir.dt.float32

_cache = {}
last_results = None


def _build_bass():
    nc = bacc.Bacc(
        "TRN2", target_bir_lowering=False, debug=False, num_devices=N_CORES
    )
    xT = nc.declare_dram_parameter("xT", [D, NTOK], BF16, isOutput=False)
    WT = [
        nc.declare_dram_parameter(f"WT{l}", [D, IDIM[l]], BF16, isOutput=False)
        for l in range(5)
    ]
    BB = nc.declare_dram_parameter("BB", [128, BBTOT], F32, isOutput=False)
    BCOL = nc.declare_dram_parameter("BCOL", [1, 8], F32, isOutput=False)
    y = nc.declare_dram_parameter("y", [BPC, I_TOTAL], F16, isOutput=True)

    xT3 = xT[:, :].rearrange("(c p) t -> p c t", p=128)      # [128, 4, NTOK]

    with TileContext(nc) as tc:
        with (
            tc.tile_pool(name="wt", bufs=1) as wt_pool,
            tc.tile_pool(name="bias", bufs=1) as bias_pool,
            tc.tile_pool(name="x", bufs=3) as x_pool,
            tc.tile_pool(name="out", bufs=4) as out_pool,
            tc.tile_pool(name="ocol", bufs=4) as ocol_pool,
            tc.tile_pool(name="ps", bufs=6, space="PSUM") as ps_pool,
            tc.tile_pool(name="pscol", bufs=2, space="PSUM") as pscol_pool,
        ):
            # Tables are loaded just-in-time per layer (first matmul would
            # otherwise stall ~35us behind 9.4 MB of upfront table DMAs).
            bb = bias_pool.tile([128, BBTOT], F32, tag="bb")
            bcol = bias_pool.tile([1, 8], F32, tag="bcol")
            nc.sync.dma_start(out=bcol[:], in_=BCOL[:, :])

            # Layer 4 early (its tiny strided stores hide under compute);
            # layer 3 last (largest, most efficient stores stream the tail).
            SEQ = [0, 4, 1, 2, 3]

            def load_tables(l):
                t = wt_pool.tile([128, 4 * IDIM[l]], BF16, tag=f"wt{l}")
                t3 = t[:].rearrange("p (c o) -> p c o", c=4)
                nc.sync.dma_start(
                    out=t3, in_=WT[l][:, :].rearrange("(c p) o -> p c o", p=128)
                )
                nc.sync.dma_start(
                    out=bb[:, BBOFF[l] : BBOFF[l] + KSCI[l]],
                    in_=BB[:, BBOFF[l] : BBOFF[l] + KSCI[l]],
                )
                return t3

            wt3_by_layer = {SEQ[0]: load_tables(SEQ[0])}
            for li, l in enumerate(SEQ):
                wt3_l = wt3_by_layer[l]
                kn, ksci = KN[l], KSCI[l]
                # y main region viewed [b, k, o]; tail region viewed [b, k]
                y_main = y[:, START[l] : START[l] + kn * ksci].rearrange(
                    "b (k o) -> b k o", o=ksci
                )
                y_col = y[:, START[l] + kn * ksci : START[l] + kn * ksci + kn]
                # subtile = whole batches when kn < 128, else 128-token slice
                ts = 128 if kn >= 128 else (128 // kn) * kn
                for t0 in range(0, TOKL[l], TLOAD):
                    tl = min(TLOAD, TOKL[l] - t0)
                    xt = x_pool.tile([128, 4 * TLOAD], BF16, tag="xt")
                    xt3 = xt[:].rearrange("p (c t) -> p c t", c=4)
                    nc.sync.dma_start(
                        out=xt3[:, :, :tl],
                        in_=xT3[:, :, XOFF[l] + t0 : XOFF[l] + t0 + tl],
                    )
                    if t0 == 0 and li + 1 < len(SEQ):
                        wt3_by_layer[SEQ[li + 1]] = load_tables(SEQ[li + 1])
                    # ---- main pass: tokens on partitions, o on free dim.
                    # All o-tiles of a token-subtile drain into one wide SBUF
                    # tile so each store DMA writes full ksci-long rows
                    # (8 KB runs for layer 3 instead of 1 KB per o-tile).
                    for s0 in range(0, tl, ts):
                        sl = min(ts, tl - s0)         # tokens in subtile
                        tok = t0 + s0                  # layer-token index
                        b0 = tok // kn                 # first batch
                        nb = max(1, sl // kn)          # batches in subtile
                        k0 = tok - b0 * kn             # first k (0 unless kn>128... )
                        ob = out_pool.tile([128, 4096], F16, tag="ob")
                        for o0 in range(0, ksci, OTILE):
                            no = min(OTILE, ksci - o0)
                            ps = ps_pool.tile([128, OTILE], F32, tag="ps")
                            for dc in range(4):
                                nc.tensor.matmul(
                                    out=ps[:sl, :no],
                                    lhsT=xt3[:, dc, s0 : s0 + sl],
                                    rhs=wt3_l[:, dc, o0 : o0 + no],
                                    start=(dc == 0),
                                    stop=(dc == 3),
                                )
                            nc.any.tensor_add(
                                out=ob[:sl, o0 : o0 + no],
                                in0=ps[:sl, :no],
                                in1=bb[:sl, BBOFF[l] + o0 : BBOFF[l] + o0 + no],
                            )
                        # store per batch: [nk, ksci] rows contiguous in y
                        nk = min(kn, sl)
                        for bi in range(nb):
                            nc.scalar.dma_start(
                                out=y_main[b0 + bi, k0 : k0 + nk, :],
                                in_=ob[bi * nk : bi * nk + nk, :ksci],
                            )
                    # ---- bias-column pass: [1, token] rows
                    for c0 in range(0, tl, OTILE):
                        cl = min(OTILE, tl - c0)
                        pc = pscol_pool.tile([1, OTILE], F32, tag="pc")
                        for dc in range(4):
                            nc.tensor.matmul(
                                out=pc[:1, :cl],
                                lhsT=wt3_l[:, dc, ksci : ksci + 1],
                                rhs=xt3[:, dc, c0 : c0 + cl],
                                start=(dc == 0),
                                stop=(dc == 3),
                            )
                        oc = ocol_pool.tile([1, OTILE], F16, tag="oc")
                        nc.any.tensor_scalar_add(
                            out=oc[:1, :cl],
                            in0=pc[:1, :cl],
                            scalar1=bcol[0:1, l : l + 1],
                        )
                        # tokens (t0+c0 .. +cl) are whole batches here
                        cb0 = (t0 + c0) // kn
                        cnb = cl // kn
                        for bi in range(cnb):
                            nc.scalar.dma_start(
                                out=y_col[cb0 + bi, :],
                                in_=oc[0:1, bi * kn : (bi + 1) * kn],
                            )
    nc.compile()
    return nc


def _prep_inputs(inputs):
    x = np.asarray(inputs["x"], dtype=np.float32)
    xb = x.astype(ml_dtypes.bfloat16)
    in_maps = []
    # shared across cores
    shared = {}
    for l in range(5):
        W = np.asarray(inputs[f"W{l}"], dtype=np.float32)
        shared[f"WT{l}"] = np.ascontiguousarray(W.astype(ml_dtypes.bfloat16).T)
    bbvec = np.concatenate(
        [np.asarray(inputs[f"b{l}"], dtype=np.float32)[: KSCI[l]] for l in range(5)]
    )
    shared["BB"] = np.ascontiguousarray(np.broadcast_to(bbvec, (128, BBTOT)))
    bcol = np.zeros((1, 8), np.float32)
    for l in range(5):
        bcol[0, l] = np.asarray(inputs[f"b{l}"], dtype=np.float32)[KSCI[l]]
    shared["BCOL"] = bcol
    off = np.cumsum([0] + KN).tolist()
    for c in range(N_CORES):
        xc = xb[c * BPC : (c + 1) * BPC]  # [16, 714, 512] bf16
        parts = [
            np.transpose(xc[:, off[l] : off[l] + KN[l]], (2, 0, 1)).reshape(D, -1)
            for l in range(5)
        ]
        xT = np.ascontiguousarray(np.concatenate(parts, axis=1))  # [512, 11424]
        in_maps.append({"xT": xT, **shared})
    return in_maps


def kernel(**inputs):
    global last_results
    if "nc" not in _cache:
        _cache["nc"] = _build_bass()
    nc = _cache["nc"]
    in_maps = _prep_inputs(inputs)
    res = run_bass_kernel_spmd(nc, in_maps, list(range(N_CORES)))
    last_results = res
    y = np.concatenate(
        [res.results[c]["y"].astype(np.float32) for c in range(N_CORES)], axis=0
    )
    return y


# revision 12
# speedup vs baseline: 1.1539x; 1.0625x over previous
"""Trainium2 Bass kernel for nn_DebedderNeuronGroup_index.

Math (per layer l, with kn=KN[l], ksci=KS[l]*CI[l], i_dim=ksci+1):
    out[b, k, o] = sum_d x[b, off_l + k, d] * W_l[o, d] + b_l[o]
    y[b, S_l + k*ksci + o] = out[b, k, o]          for o <  ksci
    y[b, S_l + kn*ksci + k] = out[b, k, ksci]      (bias column tail block)
The five layers' outputs exactly tile y's 1,422,218 columns, so every
element of y is written exactly once (pure permutation, no accumulation).

Strategy: pure data parallelism over batch (16 per core, 8 cores).
Host pre-transposes x to xT[d, token] (token order layer-major then
batch-major) and W to WT[d, o], both cast to bf16 (matmul runs 4x faster
than fp32 on the PE; rel err ~5e-4). On device, per 128-token tile:
tokens sit on PSUM partitions (stationary operand = xT tile), o on the
free dim, so every HBM store is a [tokens, o] tile whose rows are
contiguous runs in y. Bias is added during the PSUM->SBUF drain with a
host-replicated [128, ksci] broadcast table. The bias column (o == ksci)
is computed in a separate tiny pass with M=1 matmuls producing [1, token]
rows that store contiguously into the tail blocks.
"""

import numpy as np
import ml_dtypes

import concourse.bass as bass
import concourse.mybir as mybir
from concourse import bacc
from concourse.tile import TileContext
from concourse.bass_utils import run_bass_kernel_spmd

# ---------------------------------------------------------------- constants
N_CORES = 8
B = 128
BPC = B // N_CORES            # batches per core = 16
D = 512
KN = [64, 128, 256, 256, 10]
KSCI = [27, 576, 1152, 4096, 256]
IDIM = [k + 1 for k in KSCI]
START = [0, 1792, 75648, 370816, 1419648]
I_TOTAL = 1422218
TOK = sum(KN)                 # 714 tokens per batch
TOKL = [BPC * k for k in KN]  # tokens per core per layer
XOFF = np.cumsum([0] + TOKL).tolist()   # token offset per layer in xT
NTOK = XOFF[-1]               # 11424
BBOFF = np.cumsum([0] + KSCI).tolist()  # bias-broadcast offset per layer
BBTOT = BBOFF[-1]             # 6107
TLOAD = 1024                  # tokens per x DMA chunk
OTILE = 512                   # matmul moving free dim / PSUM bank
BF16 = mybir.dt.bfloat16
F16 = mybir.dt.float16
F32 = mybir.dt.float32

_cache = {}
last_results = None


def _build_bass():
    nc = bacc.Bacc(
        "TRN2", target_bir_lowering=False, debug=False, num_devices=N_CORES
    )
    xT = nc.declare_dram_parameter("xT", [D, NTOK], BF16, isOutput=False)
    WT = [
        nc.declare_dram_parameter(f"WT{l}", [D, IDIM[l]], BF16, isOutput=False)
        for l in range(5)
    ]
    BB = nc.declare_dram_parameter("BB", [128, BBTOT], F32, isOutput=False)
    BCOL = nc.declare_dram_parameter("BCOL", [1, 8], F32, isOutput=False)
    y = nc.declare_dram_parameter("y", [BPC, I_TOTAL], F16, isOutput=True)

    xT3 = xT[:, :].rearrange("(c p) t -> p c t", p=128)      # [128, 4, NTOK]

    with TileContext(nc) as tc:
        with (
            tc.tile_pool(name="wt", bufs=1) as wt_pool,
            tc.tile_pool(name="bias", bufs=1) as bias_pool,
            tc.tile_pool(name="x", bufs=3) as x_pool,
            tc.tile_pool(name="out", bufs=4) as out_pool,
            tc.tile_pool(name="ocol", bufs=4) as ocol_pool,
            tc.tile_pool(name="ps", bufs=6, space="PSUM") as ps_pool,
            tc.tile_pool(name="pscol", bufs=2, space="PSUM") as pscol_pool,
        ):
            # Tables are loaded just-in-time per layer (first matmul would
            # otherwise stall ~35us behind 9.4 MB of upfront table DMAs).
            bb = bias_pool.tile([128, BBTOT], F32, tag="bb")
            bcol = bias_pool.tile([1, 8], F32, tag="bcol")
            nc.gpsimd.dma_start(out=bcol[:], in_=BCOL[:, :])

            # Layer 4 early (its tiny strided stores hide under compute);
            # layer 3 last (largest, most efficient stores stream the tail).
            SEQ = [0, 4, 1, 2, 3]

            def load_tables(l):
                t = wt_pool.tile([128, 4 * IDIM[l]], BF16, tag=f"wt{l}")
                t3 = t[:].rearrange("p (c o) -> p c o", c=4)
                nc.gpsimd.dma_start(
                    out=t3, in_=WT[l][:, :].rearrange("(c p) o -> p c o", p=128)
                )
                nc.gpsimd.dma_start(
                    out=bb[:, BBOFF[l] : BBOFF[l] + KSCI[l]],
                    in_=BB[:, BBOFF[l] : BBOFF[l] + KSCI[l]],
                )
                return t3

            wt3_by_layer = {SEQ[0]: load_tables(SEQ[0])}
            for li, l in enumerate(SEQ):
                wt3_l = wt3_by_layer[l]
                kn, ksci = KN[l], KSCI[l]
                # y main region viewed [b, k, o]; tail region viewed [b, k]
                y_main = y[:, START[l] : START[l] + kn * ksci].rearrange(
                    "b (k o) -> b k o", o=ksci
                )
                y_col = y[:, START[l] + kn * ksci : START[l] + kn * ksci + kn]
                # subtile = whole batches when kn < 128, else 128-token slice
                ts = 128 if kn >= 128 else (128 // kn) * kn
                for t0 in range(0, TOKL[l], TLOAD):
                    tl = min(TLOAD, TOKL[l] - t0)
                    xt = x_pool.tile([128, 4 * TLOAD], BF16, tag="xt")
                    xt3 = xt[:].rearrange("p (c t) -> p c t", c=4)
                    nc.sync.dma_start(
                        out=xt3[:, :, :tl],
                        in_=xT3[:, :, XOFF[l] + t0 : XOFF[l] + t0 + tl],
                    )
                    if t0 == 0 and li + 1 < len(SEQ):
                        wt3_by_layer[SEQ[li + 1]] = load_tables(SEQ[li + 1])
                    # ---- main pass: tokens on partitions, o on free dim.
                    # All o-tiles of a token-subtile drain into one wide SBUF
                    # tile so each store DMA writes full ksci-long rows
                    # (8 KB runs for layer 3 instead of 1 KB per o-tile).
                    for s0 in range(0, tl, ts):
                        sl = min(ts, tl - s0)         # tokens in subtile
                        tok = t0 + s0                  # layer-token index
                        b0 = tok // kn                 # first batch
                        nb = max(1, sl // kn)          # batches in subtile
                        k0 = tok - b0 * kn             # first k (0 unless kn>128... )
                        ob = out_pool.tile([128, 4096], F16, tag="ob")
                        for o0 in range(0, ksci, OTILE):
                            no = min(OTILE, ksci - o0)
                            ps = ps_pool.tile([128, OTILE], F32, tag="ps")
                            for dc in range(4):
                                nc.tensor.matmul(
                                    out=ps[:sl, :no],
                                    lhsT=xt3[:, dc, s0 : s0 + sl],
                                    rhs=wt3_l[:, dc, o0 : o0 + no],
                                    start=(dc == 0),
                                    stop=(dc == 3),
                                )
                            nc.any.tensor_add(
                                out=ob[:sl, o0 : o0 + no],
                                in0=ps[:sl, :no],
                                in1=bb[:sl, BBOFF[l] + o0 : BBOFF[l] + o0 + no],
                            )
                        # store per batch: [nk, ksci] rows contiguous in y
                        nk = min(kn, sl)
                        for bi in range(nb):
                            nc.scalar.dma_start(
                                out=y_main[b0 + bi, k0 : k0 + nk, :],
                                in_=ob[bi * nk : bi * nk + nk, :ksci],
                            )
                    # ---- bias-column pass: [1, token] rows
                    for c0 in range(0, tl, OTILE):
                        cl = min(OTILE, tl - c0)
                        pc = pscol_pool.tile([1, OTILE], F32, tag="pc")
                        for dc in range(4):
                            nc.tensor.matmul(
                                out=pc[:1, :cl],
                                lhsT=wt3_l[:, dc, ksci : ksci + 1],
                                rhs=xt3[:, dc, c0 : c0 + cl],
                                start=(dc == 0),
                                stop=(dc == 3),
                            )
                        oc = ocol_pool.tile([1, OTILE], F16, tag="oc")
                        nc.any.tensor_scalar_add(
                            out=oc[:1, :cl],
                            in0=pc[:1, :cl],
                            scalar1=bcol[0:1, l : l + 1],
                        )
                        # tokens (t0+c0 .. +cl) are whole batches here
                        cb0 = (t0 + c0) // kn
                        cnb = cl // kn
                        for bi in range(cnb):
                            nc.gpsimd.dma_start(
                                out=y_col[cb0 + bi, :],
                                in_=oc[0:1, bi * kn : (bi + 1) * kn],
                            )
    nc.compile()
    return nc


def _prep_inputs(inputs):
    x = np.asarray(inputs["x"], dtype=np.float32)
    xb = x.astype(ml_dtypes.bfloat16)
    in_maps = []
    # shared across cores
    shared = {}
    for l in range(5):
        W = np.asarray(inputs[f"W{l}"], dtype=np.float32)
        shared[f"WT{l}"] = np.ascontiguousarray(W.astype(ml_dtypes.bfloat16).T)
    bbvec = np.concatenate(
        [np.asarray(inputs[f"b{l}"], dtype=np.float32)[: KSCI[l]] for l in range(5)]
    )
    shared["BB"] = np.ascontiguousarray(np.broadcast_to(bbvec, (128, BBTOT)))
    bcol = np.zeros((1, 8), np.float32)
    for l in range(5):
        bcol[0, l] = np.asarray(inputs[f"b{l}"], dtype=np.float32)[KSCI[l]]
    shared["BCOL"] = bcol
    off = np.cumsum([0] + KN).tolist()
    for c in range(N_CORES):
        xc = xb[c * BPC : (c + 1) * BPC]  # [16, 714, 512] bf16
        parts = [
            np.transpose(xc[:, off[l] : off[l] + KN[l]], (2, 0, 1)).reshape(D, -1)
            for l in range(5)
        ]
        xT = np.ascontiguousarray(np.concatenate(parts, axis=1))  # [512, 11424]
        in_maps.append({"xT": xT, **shared})
    return in_maps


def kernel(**inputs):
    global last_results
    if "nc" not in _cache:
        _cache["nc"] = _build_bass()
    nc = _cache["nc"]
    in_maps = _prep_inputs(inputs)
    res = run_bass_kernel_spmd(nc, in_maps, list(range(N_CORES)))
    last_results = res
    y = np.concatenate(
        [res.results[c]["y"].astype(np.float32) for c in range(N_CORES)], axis=0
    )
    return y
